# revision 7
# baseline (speedup 1.0000x reference)
"""Trainium2 Bass kernel for the 2-layer-summed GCN message-passing problem.

Math (from the reference):
  g1 = 2*(relu(spmm(adj1_0, e1)) + relu(spmm(adj1_1, e1)))   e1=[dEmbed;mEmbed]
  g2 = 2*(relu(spmm(adj2_0, e2)) + relu(spmm(adj2_1, e2)))   e2=[pEmbed;mEmbed]
  outputs: mEmbed_out = inter*g1[d:] + (1-inter)*g2[p:], dEmbed_gcn = g1[:d],
           pEmbed_gcn = g2[:p]
(leakyrelu(0.5) followed by relu == relu; the double loop doubles each term.)

Strategy: 1D partition of destination rows across 8 cores (d/m/p row ranges
split evenly). Host sorts/buckets edges per core into 128-row destination
blocks (round-robin degree packing), fp16 embedding tables are gathered
edge-wise with dma_gather (int16 chunk-local indices), a per-tile scaled
selection matrix S_T[e,r] = val[e]*(dest[e]==r) is built with one DVE op, and
the segment-sum is a chain of fp16 PE matmuls accumulating in PSUM. ReLU (+
runtime `inter` scale for m-blocks) on ACT, A+B add on DVE, then
dma_scatter_add writes each block's 128 rows to the per-core output slice.
"""
import sys

sys.path.insert(0, "/opt/trn_rl_repo")

import numpy as np

DIAG, MED, PRO, FD = 50000, 20000, 40000, 128
N1, N2 = DIAG + MED, PRO + MED
NCORES = 8
ND, NM, NP = DIAG // NCORES, MED // NCORES, PRO // NCORES  # 6250, 2500, 5000

CHB1 = [0, 23334, 46667, N1]   # e1 source chunks (int16 gather index limit)
CHB2 = [0, 30000, N2]          # e2 source chunks
T1, T2 = 6, 9                  # gather tiles per (block, chunk) -> 18 tiles/blk
NBD, NBM = (ND + 127) // 128, (NM + 127) // 128   # 49, 20
NBP = (NP + 127) // 128                            # 40
WG = 4                         # blocks per gather/scatter window


def _wrap16(L):
    """Index list -> dma_gather/scatter idx layout [128, n/16] (int16)."""
    w = np.asarray(L, np.int16).reshape(-1, 16).T
    return np.tile(w, (8, 1))


def _pack_rows(deg, nb, cap):
    """Assign nrows rows to nb blocks (<=128 rows each) s.t. per-(spmm,chunk)
    degree sums stay <= cap. deg: [nrows, nspmm, nch]. Returns block id/slot."""
    nrows = deg.shape[0]
    tot = deg.sum(axis=(1, 2))
    order = np.argsort(-tot, kind="stable")
    block = np.empty(nrows, np.int32)
    block[order] = np.arange(nrows, dtype=np.int32) % nb
    # repair per-(block, spmm, chunk) cap violations by moving rows
    nsp, nch = deg.shape[1], deg.shape[2]
    for _ in range(2000):
        sums = np.zeros((nb, nsp, nch), np.int64)
        for s in range(nsp):
            for c in range(nch):
                sums[:, s, c] = np.bincount(block, weights=deg[:, s, c],
                                            minlength=nb)
        counts = np.bincount(block, minlength=nb)
        viol = np.argwhere(sums > cap)
        if len(viol) == 0:
            break
        b, s, c = viol[0]
        rows_b = np.where(block == b)[0]
        mover = rows_b[np.argmax(deg[rows_b, s, c])]
        # target: block with room and smallest sum in the violated (s, c)
        cand = np.where(counts < 128)[0]
        cand = cand[cand != b]
        assert len(cand) > 0, "no block with free slot"
        tgt = cand[np.argmin(sums[cand, s, c])]
        block[mover] = tgt
    else:
        raise AssertionError("packing repair did not converge")
    # slots: position within block
    slot = np.empty(nrows, np.int32)
    for b in range(nb):
        rows_b = np.where(block == b)[0]
        assert len(rows_b) <= 128
        slot[rows_b] = np.arange(len(rows_b), dtype=np.int32)
    return block, slot


def _windows(nb_d, nb_m):
    """[(phase, [block ids])] with phase 0 = primary (d/p), 1 = m."""
    out = []
    for ph, lo, hi in ((0, 0, nb_d), (1, nb_d, nb_d + nb_m)):
        b = lo
        while b < hi:
            out.append((ph, list(range(b, min(b + WG, hi)))))
            b += WG
    return out


def _prep_pair(rows_l, cols_l, vals_l, prim, nprim_l, nm_l, chb, tpc, nb_p, nb_m):
    """Host prep for one spmm pair (2 adjacency slices) for all cores.

    rows_l/cols_l/vals_l: lists of 2 arrays (the two adjacency slices).
    prim: size of the primary segment (DIAG or PRO) in the concat table.
    nprim_l/nm_l: per-core primary/m row counts. chb: chunk boundaries.
    Returns per-core dict of device arrays + stream geometry.
    """
    nch = len(chb) - 1
    nb = nb_p + nb_m
    ntile_blk = 2 * nch * tpc
    wins = _windows(nb_p, nb_m)

    rows = np.concatenate(rows_l)
    cols = np.concatenate(cols_l)
    vals = np.concatenate(vals_l).astype(np.float32) * 2.0
    sarr = np.concatenate([np.zeros(len(rows_l[0]), np.int8),
                           np.ones(len(rows_l[1]), np.int8)])

    is_m = rows >= prim
    m_idx = rows - prim
    core = np.where(is_m, m_idx // nm_l, rows // nprim_l).astype(np.int32)
    lrow = np.where(is_m, m_idx % nm_l, rows % nprim_l).astype(np.int32)
    # local row id within core's (primary + m) group: primary rows then m rows
    grow = np.where(is_m, nprim_l + lrow, lrow).astype(np.int32)
    bnd = np.asarray(chb[1:-1])
    chunk = np.searchsorted(bnd, cols, side="right").astype(np.int32)
    lcol = (cols - np.asarray(chb)[chunk]).astype(np.int32)

    per_core = []
    nrows_core = nprim_l + nm_l
    for cid in range(NCORES):
        sel = core == cid
        g, s, c, lc, v = (grow[sel], sarr[sel], chunk[sel], lcol[sel],
                          vals[sel])
        # degrees [nrows, 2, nch]
        deg = np.zeros((nrows_core, 2, nch), np.int32)
        np.add.at(deg, (g, s.astype(np.int64), c), 1)
        # pack primary rows into nb_p blocks, m rows into nb_m blocks
        bp, sp_ = _pack_rows(deg[:nprim_l], nb_p, tpc * 128)
        bm, sm_ = _pack_rows(deg[nprim_l:], nb_m, tpc * 128)
        blk_of = np.concatenate([bp, nb_p + bm])
        slot_of = np.concatenate([sp_, sm_])
        eb = blk_of[g]
        eslot = slot_of[g]
        # group edges by (spmm, block, chunk); place into padded layout
        key = (s.astype(np.int64) * nb + eb) * nch + c
        order = np.argsort(key, kind="stable")
        key_s = key[order]
        grp, cnt = np.unique(key_s, return_counts=True)
        assert cnt.max() <= tpc * 128, f"group overflow {cnt.max()}"
        rank = np.arange(len(key_s)) - np.repeat(
            np.cumsum(cnt) - cnt, cnt)
        pos = key_s * (tpc * 128) + rank
        tot_slots = 2 * nb * nch * tpc * 128
        p_idx = np.zeros(tot_slots, np.int16)
        p_val = np.zeros(tot_slots, np.float16)
        p_dst = np.zeros(tot_slots, np.float16)
        p_idx[pos] = lc[order].astype(np.int16)
        p_val[pos] = v[order].astype(np.float16)
        p_dst[pos] = eslot[order].astype(np.float16)
        # padded tile view: [s, b, c, t, 128]
        p_idx = p_idx.reshape(2, nb, nch, tpc, 128)
        p_val = p_val.reshape(2, nb, nch, tpc, 128)
        p_dst = p_dst.reshape(2, nb, nch, tpc, 128)

        # device stream order: for window: for chunk: for s: for b in w: for t
        gidx_slabs, dest_cols, vals_cols = [], [], []
        for ph, blks in wins:
            for ci in range(nch):
                for s2 in range(2):
                    for b in blks:
                        for t in range(tpc):
                            gidx_slabs.append(_wrap16(p_idx[s2, b, ci, t]))
                            dest_cols.append(p_dst[s2, b, ci, t])
                            vals_cols.append(p_val[s2, b, ci, t])
        gidx = np.concatenate(gidx_slabs, axis=1)          # [128, ntiles*8]
        dest = np.stack(dest_cols, axis=1)                 # [128, ntiles]
        valsd = np.stack(vals_cols, axis=1)
        # scatter idx: per block, 128 tokens -> local out row. Empty slots
        # point at a dummy row (nprim_l / nm_l) so their +0 RMW can't race a
        # real row's accumulation inside/across dma_scatter_add instructions.
        srows = np.zeros((nb, 128), np.int16)
        for b in range(nb):
            if b < nb_p:
                srows[b, :] = nprim_l
                rws = np.where(bp == b)[0]
                srows[b, sp_[rws]] = rws.astype(np.int16)
            else:
                srows[b, :] = nm_l
                rws = np.where(bm == (b - nb_p))[0]
                srows[b, sm_[rws]] = rws.astype(np.int16)
        sidx = np.concatenate([_wrap16(srows[b]) for b in range(nb)], axis=1)
        per_core.append(dict(gidx=gidx, dest=dest, vals=valsd, sidx=sidx))
    return per_core, wins, nch, ntile_blk


def _build_program(geo1, geo2, tc1, tc2):
    from concourse import bass, bacc, mybir, tile

    f16, f32, i16 = mybir.dt.float16, mybir.dt.float32, mybir.dt.int16
    nc = bacc.Bacc("TRN2", target_bir_lowering=False, debug=False)

    e1 = nc.dram_tensor("e1", [N1, FD], f16, kind="ExternalInput")
    e2 = nc.dram_tensor("e2", [N2, FD], f16, kind="ExternalInput")
    ins = {}
    for nm_, tcn, nbt in (("g1", tc1, NBD + NBM), ("g2", tc2, NBP + NBM)):
        ins[nm_ + "_gidx"] = nc.dram_tensor(nm_ + "_gidx", [128, tcn * 8], i16,
                                            kind="ExternalInput")
        ins[nm_ + "_dest"] = nc.dram_tensor(nm_ + "_dest", [128, tcn], f16,
                                            kind="ExternalInput")
        ins[nm_ + "_vals"] = nc.dram_tensor(nm_ + "_vals", [128, tcn], f16,
                                            kind="ExternalInput")
        ins[nm_ + "_sidx"] = nc.dram_tensor(nm_ + "_sidx", [128, nbt * 8], i16,
                                            kind="ExternalInput")
    inter_d = nc.dram_tensor("inter_t", [128, 1], f32, kind="ExternalInput")
    ointer_d = nc.dram_tensor("ointer_t", [128, 1], f32, kind="ExternalInput")

    # +1 dummy row absorbs empty-slot scatter tokens (stripped on host)
    out_d = nc.dram_tensor("out_d", [ND + 1, FD], f32, kind="ExternalOutput")
    out_m = nc.dram_tensor("out_m", [NM + 1, FD], f32, kind="ExternalOutput")
    out_p = nc.dram_tensor("out_p", [NP + 1, FD], f32, kind="ExternalOutput")

    wins1, nch1 = geo1
    wins2, nch2 = geo2

    with tile.TileContext(nc) as tc:
        with (
            tc.tile_pool(name="const", bufs=1) as constp,
            tc.tile_pool(name="meta", bufs=1) as metap,
            tc.tile_pool(name="gidx", bufs=3) as gidxp,
            tc.tile_pool(name="msgs", bufs=2) as msgsp,
            tc.tile_pool(name="s", bufs=4) as sp,
            tc.tile_pool(name="res", bufs=4) as resp,
            tc.tile_pool(name="stage", bufs=2) as stagep,
            tc.tile_pool(name="psum", bufs=4, space="PSUM") as psump,
        ):
            iota_t = constp.tile([128, 128], f16)
            nc.gpsimd.iota(iota_t[:], pattern=[[1, 128]], base=0,
                           channel_multiplier=0,
                           allow_small_or_imprecise_dtypes=True)
            inter_t = constp.tile([128, 1], f32)
            nc.sync.dma_start(inter_t[:], inter_d[:])
            ointer_t = constp.tile([128, 1], f32)
            nc.sync.dma_start(ointer_t[:], ointer_d[:])

            for pair, (wins, nch, tpc, table, chb, tcn, pout, mscale) in enumerate((
                ("g1", nch1, T1, e1, CHB1, tc1, out_d, inter_t),
                ("g2", nch2, T2, e2, CHB2, tc2, out_p, ointer_t),
            )):
                nm_ = "g1" if pair == 0 else "g2"
                wins = wins1 if pair == 0 else wins2
                nbt = (NBD + NBM) if pair == 0 else (NBP + NBM)
                dest_t = metap.tile([128, tcn], f16, tag="dest")
                nc.sync.dma_start(dest_t[:], ins[nm_ + "_dest"][:])
                vals_t = metap.tile([128, tcn], f16, tag="vals")
                nc.sync.dma_start(vals_t[:], ins[nm_ + "_vals"][:])
                sidx_t = metap.tile([128, nbt * 8], i16, tag="sidx")
                nc.sync.dma_start(sidx_t[:], ins[nm_ + "_sidx"][:])

                tile_col = 0  # global tile counter (matches host stream order)
                for ph, blks in wins:
                    wb = len(blks)
                    ntw = 2 * nch * tpc * wb           # tiles in this window
                    gslab = gidxp.tile([128, ntw * 8], i16, tag="gidx")
                    nc.sync.dma_start(
                        gslab[:], ins[nm_ + "_gidx"][:, tile_col * 8:
                                                     (tile_col + ntw) * 8])
                    msgs = msgsp.tile([128, ntw, FD], f16, tag="msgs")
                    npc = 2 * tpc * wb                 # tiles per chunk gather
                    for ci in range(nch):
                        nidx = npc * 128
                        nc.gpsimd.dma_gather(
                            msgs[:, ci * npc:(ci + 1) * npc, :],
                            table[chb[ci]:chb[ci + 1], :],
                            gslab[:, ci * npc * 8:(ci + 1) * npc * 8],
                            nidx, nidx, FD, single_packet=False)
                    stage = stagep.tile([128, wb, FD], f32, tag="stage")
                    for bi in range(wb):
                        res = [None, None]
                        for s2 in range(2):
                            acc = psump.tile([128, FD], f32, space="PSUM",
                                             tag="acc")
                            k = 0
                            for ci in range(nch):
                                for t in range(tpc):
                                    g = (ci * npc + s2 * tpc * wb
                                         + bi * tpc + t)
                                    col = tile_col + g
                                    s_t = sp.tile([128, 128], f16, tag="s_t")
                                    nc.vector.scalar_tensor_tensor(
                                        out=s_t[:],
                                        in0=iota_t[:],
                                        scalar=dest_t[:, col:col + 1],
                                        in1=vals_t[:, col:col + 1]
                                            .to_broadcast([128, 128]),
                                        op0=mybir.AluOpType.is_equal,
                                        op1=mybir.AluOpType.mult,
                                    )
                                    nc.tensor.matmul(
                                        out=acc[:], lhsT=s_t[:],
                                        rhs=msgs[:, g, :],
                                        start=(k == 0),
                                        stop=(k == nch * tpc - 1))
                                    k += 1
                            r = resp.tile([128, FD], f32, tag=f"res{s2}")
                            if ph == 0:
                                nc.scalar.activation(
                                    out=r[:], in_=acc[:],
                                    func=mybir.ActivationFunctionType.Relu)
                            else:
                                nc.scalar.activation(
                                    out=r[:], in_=acc[:],
                                    func=mybir.ActivationFunctionType.Relu,
                                    scale=mscale[:, :1])
                            res[s2] = r
                        nc.vector.tensor_tensor(
                            out=stage[:, bi, :], in0=res[0][:], in1=res[1][:],
                            op=mybir.AluOpType.add)
                    tgt = pout if ph == 0 else out_m
                    b0 = blks[0]
                    nc.gpsimd.dma_scatter_add(
                        tgt[:], stage[:, 0:wb, :],
                        sidx_t[:, b0 * 8:(b0 + wb) * 8],
                        wb * 128, wb * 128, FD)
                    tile_col += ntw
    nc.compile()
    return nc


def _install_ntff_shim():
    """Provide antenv.axon_hooks (missing in this image) so trace=True works."""
    import types, ctypes, contextlib
    try:
        from antenv.axon_hooks import get_axon_ntff_profile_hook  # noqa
        return
    except ImportError:
        pass
    so_path = "/opt/axon/libaxon_pjrt.so"
    try:
        lib = ctypes.CDLL(so_path)
        assert hasattr(lib, "axon_start_nrt_profile")
    except Exception:
        lib = None
    if lib is not None:
        lib.axon_start_nrt_profile.argtypes = [ctypes.POINTER(ctypes.c_int64),
                                               ctypes.c_size_t]
        lib.axon_start_nrt_profile.restype = ctypes.c_int64
        lib.axon_stop_nrt_profile.argtypes = [ctypes.c_char_p]
        lib.axon_stop_nrt_profile.restype = ctypes.c_int64

        @contextlib.contextmanager
        def _hook(output_dir, device_ids):
            import jax
            jax.devices()
            if device_ids:
                ids = (ctypes.c_int64 * len(device_ids))(*device_ids)
                rc = lib.axon_start_nrt_profile(ids, len(device_ids))
            else:
                rc = lib.axon_start_nrt_profile(None, 0)
            if rc != 0:
                raise RuntimeError(f"axon_start_nrt_profile rc={rc}")
            try:
                yield
            finally:
                n = lib.axon_stop_nrt_profile(str(output_dir).encode())
                print(f"ntff profile: {n} file(s) -> {output_dir}")
    else:
        _hook = None
    mod = types.ModuleType("antenv.axon_hooks")
    mod.get_axon_ntff_profile_hook = lambda: _hook
    mod.set_axon_ntff_profile_hook = lambda h: None
    import antenv
    sys.modules["antenv.axon_hooks"] = mod
    antenv.axon_hooks = mod


def kernel(adj1_rows, adj1_cols, adj1_vals, adj2_rows, adj2_cols, adj2_vals,
           dEmbed, mEmbed, pEmbed, inter, _trace=False):
    from concourse.bass_utils import run_bass_kernel_spmd
    if _trace:
        _install_ntff_shim()

    adj1_rows, adj1_cols = np.asarray(adj1_rows), np.asarray(adj1_cols)
    adj1_vals = np.asarray(adj1_vals)
    adj2_rows, adj2_cols = np.asarray(adj2_rows), np.asarray(adj2_cols)
    adj2_vals = np.asarray(adj2_vals)
    dEmbed = np.asarray(dEmbed, np.float32)
    mEmbed = np.asarray(mEmbed, np.float32)
    pEmbed = np.asarray(pEmbed, np.float32)
    inter_v = float(np.asarray(inter).reshape(-1)[0])

    e1 = np.concatenate([dEmbed, mEmbed], axis=0).astype(np.float16)
    e2 = np.concatenate([pEmbed, mEmbed], axis=0).astype(np.float16)

    pc1, wins1, nch1, _ = _prep_pair(
        [adj1_rows[0], adj1_rows[1]], [adj1_cols[0], adj1_cols[1]],
        [adj1_vals[0], adj1_vals[1]], DIAG, ND, NM, CHB1, T1, NBD, NBM)
    pc2, wins2, nch2, _ = _prep_pair(
        [adj2_rows[0], adj2_rows[1]], [adj2_cols[0], adj2_cols[1]],
        [adj2_vals[0], adj2_vals[1]], PRO, NP, NM, CHB2, T2, NBP, NBM)

    tc1 = pc1[0]["dest"].shape[1]
    tc2 = pc2[0]["dest"].shape[1]
    nc = _build_program((wins1, nch1), (wins2, nch2), tc1, tc2)

    inter_t = np.full((128, 1), inter_v, np.float32)
    ointer_t = np.full((128, 1), 1.0 - inter_v, np.float32)
    in_maps = []
    for cid in range(NCORES):
        in_maps.append({
            "e1": e1, "e2": e2,
            "g1_gidx": pc1[cid]["gidx"], "g1_dest": pc1[cid]["dest"],
            "g1_vals": pc1[cid]["vals"], "g1_sidx": pc1[cid]["sidx"],
            "g2_gidx": pc2[cid]["gidx"], "g2_dest": pc2[cid]["dest"],
            "g2_vals": pc2[cid]["vals"], "g2_sidx": pc2[cid]["sidx"],
            "inter_t": inter_t, "ointer_t": ointer_t,
        })

    r = run_bass_kernel_spmd(nc, in_maps, core_ids=list(range(NCORES)),
                             trace=_trace)
    outs = r.results
    dE = np.concatenate([outs[c]["out_d"][:ND] for c in range(NCORES)], axis=0)
    mE = np.concatenate([outs[c]["out_m"][:NM] for c in range(NCORES)], axis=0)
    pE = np.concatenate([outs[c]["out_p"][:NP] for c in range(NCORES)], axis=0)
    if _trace:
        return (mE, dE, pE), r
    return (mE, dE, pE)


# revision 11
# speedup vs baseline: 2.2738x; 2.2738x over previous
"""Trainium2 Bass kernel for the 2-layer-summed GCN message-passing problem.

Math (from the reference):
  g1 = 2*(relu(spmm(adj1_0, e1)) + relu(spmm(adj1_1, e1)))   e1=[dEmbed;mEmbed]
  g2 = 2*(relu(spmm(adj2_0, e2)) + relu(spmm(adj2_1, e2)))   e2=[pEmbed;mEmbed]
  outputs: mEmbed_out = inter*g1[d:] + (1-inter)*g2[p:], dEmbed_gcn = g1[:d],
           pEmbed_gcn = g2[:p]
(leakyrelu(0.5) followed by relu == relu; the double loop doubles each term.)

Strategy: 1D partition of destination rows across 8 cores (d/m/p row ranges
split evenly). Host sorts/buckets edges per core into 128-row destination
blocks (round-robin degree packing), fp16 embedding tables are gathered
edge-wise with dma_gather (int16 chunk-local indices), a per-tile scaled
selection matrix S_T[e,r] = val[e]*(dest[e]==r) is built with one DVE op, and
the segment-sum is a chain of fp16 PE matmuls accumulating in PSUM. ReLU (+
runtime `inter` scale for m-blocks) on ACT, A+B add on DVE, then
dma_scatter_add writes each block's 128 rows to the per-core output slice.
"""
import sys

sys.path.insert(0, "/opt/trn_rl_repo")

import numpy as np

DIAG, MED, PRO, FD = 50000, 20000, 40000, 128
N1, N2 = DIAG + MED, PRO + MED
NCORES = 8
ND, NM, NP = DIAG // NCORES, MED // NCORES, PRO // NCORES  # 6250, 2500, 5000

CHB1 = [0, 23334, 46667, N1]   # e1 source chunks (int16 gather index limit)
CHB2 = [0, 30000, N2]          # e2 source chunks
T1, T2 = 6, 9                  # gather tiles per (block, chunk) -> 18 tiles/blk
NBD, NBM = (ND + 127) // 128, (NM + 127) // 128   # 49, 20
NBP = (NP + 127) // 128                            # 40
WG = 4                         # blocks per gather/scatter window


def _wrap16(L):
    """Index list -> dma_gather/scatter idx layout [128, n/16] (int16)."""
    w = np.asarray(L, np.int16).reshape(-1, 16).T
    return np.tile(w, (8, 1))


def _pack_rows(deg, nb, cap):
    """Assign nrows rows to nb blocks (<=128 rows each) s.t. per-(spmm,chunk)
    degree sums stay <= cap. deg: [nrows, nspmm, nch]. Returns block id/slot."""
    nrows = deg.shape[0]
    tot = deg.sum(axis=(1, 2))
    order = np.argsort(-tot, kind="stable")
    block = np.empty(nrows, np.int32)
    block[order] = np.arange(nrows, dtype=np.int32) % nb
    # repair per-(block, spmm, chunk) cap violations by moving rows
    nsp, nch = deg.shape[1], deg.shape[2]
    for _ in range(2000):
        sums = np.zeros((nb, nsp, nch), np.int64)
        for s in range(nsp):
            for c in range(nch):
                sums[:, s, c] = np.bincount(block, weights=deg[:, s, c],
                                            minlength=nb)
        counts = np.bincount(block, minlength=nb)
        viol = np.argwhere(sums > cap)
        if len(viol) == 0:
            break
        b, s, c = viol[0]
        rows_b = np.where(block == b)[0]
        mover = rows_b[np.argmax(deg[rows_b, s, c])]
        # target: block with room and smallest sum in the violated (s, c)
        cand = np.where(counts < 128)[0]
        cand = cand[cand != b]
        assert len(cand) > 0, "no block with free slot"
        tgt = cand[np.argmin(sums[cand, s, c])]
        block[mover] = tgt
    else:
        raise AssertionError("packing repair did not converge")
    # slots: position within block
    slot = np.empty(nrows, np.int32)
    for b in range(nb):
        rows_b = np.where(block == b)[0]
        assert len(rows_b) <= 128
        slot[rows_b] = np.arange(len(rows_b), dtype=np.int32)
    return block, slot


def _windows(nb_d, nb_m):
    """[(phase, [block ids])] with phase 0 = primary (d/p), 1 = m."""
    out = []
    for ph, lo, hi in ((0, 0, nb_d), (1, nb_d, nb_d + nb_m)):
        b = lo
        while b < hi:
            out.append((ph, list(range(b, min(b + WG, hi)))))
            b += WG
    return out


def _prep_pair(rows_l, cols_l, vals_l, prim, nprim_l, nm_l, chb, tpc, nb_p, nb_m):
    """Host prep for one spmm pair (2 adjacency slices) for all cores.

    rows_l/cols_l/vals_l: lists of 2 arrays (the two adjacency slices).
    prim: size of the primary segment (DIAG or PRO) in the concat table.
    nprim_l/nm_l: per-core primary/m row counts. chb: chunk boundaries.
    Returns per-core dict of device arrays + stream geometry.
    """
    nch = len(chb) - 1
    nb = nb_p + nb_m
    ntile_blk = 2 * nch * tpc
    wins = _windows(nb_p, nb_m)

    rows = np.concatenate(rows_l)
    cols = np.concatenate(cols_l)
    vals = np.concatenate(vals_l).astype(np.float32) * 2.0
    sarr = np.concatenate([np.zeros(len(rows_l[0]), np.int8),
                           np.ones(len(rows_l[1]), np.int8)])

    is_m = rows >= prim
    m_idx = rows - prim
    core = np.where(is_m, m_idx // nm_l, rows // nprim_l).astype(np.int32)
    lrow = np.where(is_m, m_idx % nm_l, rows % nprim_l).astype(np.int32)
    # local row id within core's (primary + m) group: primary rows then m rows
    grow = np.where(is_m, nprim_l + lrow, lrow).astype(np.int32)
    bnd = np.asarray(chb[1:-1])
    chunk = np.searchsorted(bnd, cols, side="right").astype(np.int32)
    lcol = (cols - np.asarray(chb)[chunk]).astype(np.int32)

    per_core = []
    nrows_core = nprim_l + nm_l
    for cid in range(NCORES):
        sel = core == cid
        g, s, c, lc, v = (grow[sel], sarr[sel], chunk[sel], lcol[sel],
                          vals[sel])
        # degrees [nrows, 2, nch]
        deg = np.zeros((nrows_core, 2, nch), np.int32)
        np.add.at(deg, (g, s.astype(np.int64), c), 1)
        # pack primary rows into nb_p blocks, m rows into nb_m blocks
        bp, sp_ = _pack_rows(deg[:nprim_l], nb_p, tpc * 128)
        bm, sm_ = _pack_rows(deg[nprim_l:], nb_m, tpc * 128)
        blk_of = np.concatenate([bp, nb_p + bm])
        slot_of = np.concatenate([sp_, sm_])
        eb = blk_of[g]
        eslot = slot_of[g]
        # group edges by (spmm, block, chunk); place into padded layout
        key = (s.astype(np.int64) * nb + eb) * nch + c
        order = np.argsort(key, kind="stable")
        key_s = key[order]
        grp, cnt = np.unique(key_s, return_counts=True)
        assert cnt.max() <= tpc * 128, f"group overflow {cnt.max()}"
        rank = np.arange(len(key_s)) - np.repeat(
            np.cumsum(cnt) - cnt, cnt)
        pos = key_s * (tpc * 128) + rank
        tot_slots = 2 * nb * nch * tpc * 128
        p_idx = np.zeros(tot_slots, np.int16)
        p_val = np.zeros(tot_slots, np.float16)
        p_dst = np.zeros(tot_slots, np.float16)
        p_idx[pos] = lc[order].astype(np.int16)
        p_val[pos] = v[order].astype(np.float16)
        p_dst[pos] = eslot[order].astype(np.float16)
        # padded tile view: [s, b, c, t, 128]
        p_idx = p_idx.reshape(2, nb, nch, tpc, 128)
        p_val = p_val.reshape(2, nb, nch, tpc, 128)
        p_dst = p_dst.reshape(2, nb, nch, tpc, 128)

        # device stream order: for window: for chunk: for s: for b in w: for t
        gidx_slabs, dest_cols, vals_cols = [], [], []
        for ph, blks in wins:
            for ci in range(nch):
                for s2 in range(2):
                    for b in blks:
                        for t in range(tpc):
                            gidx_slabs.append(_wrap16(p_idx[s2, b, ci, t]))
                            dest_cols.append(p_dst[s2, b, ci, t])
                            vals_cols.append(p_val[s2, b, ci, t])
        gidx = np.concatenate(gidx_slabs, axis=1)          # [128, ntiles*8]
        dest = np.stack(dest_cols, axis=1)                 # [128, ntiles]
        valsd = np.stack(vals_cols, axis=1)
        # scatter idx: per block, 128 tokens -> local out row. Empty slots
        # point at a dummy row (nprim_l / nm_l) so their +0 RMW can't race a
        # real row's accumulation inside/across dma_scatter_add instructions.
        srows = np.zeros((nb, 128), np.int16)
        for b in range(nb):
            if b < nb_p:
                srows[b, :] = nprim_l
                rws = np.where(bp == b)[0]
                srows[b, sp_[rws]] = rws.astype(np.int16)
            else:
                srows[b, :] = nm_l
                rws = np.where(bm == (b - nb_p))[0]
                srows[b, sm_[rws]] = rws.astype(np.int16)
        sidx = np.concatenate([_wrap16(srows[b]) for b in range(nb)], axis=1)
        per_core.append(dict(gidx=gidx, dest=dest, vals=valsd, sidx=sidx))
    return per_core, wins, nch, ntile_blk


def _build_program(geo1, geo2, tc1, tc2):
    from concourse import bass, bacc, mybir, tile

    f16, f32, i16 = mybir.dt.float16, mybir.dt.float32, mybir.dt.int16
    # 4 SWDGE queues: dma_gather/dma_scatter_add descriptor generation runs on
    # Q7 core-pair `queue_num` — round-robining queues parallelizes desc-gen 4x.
    nc = bacc.Bacc("TRN2", target_bir_lowering=False, debug=False,
                   num_swdge_queues=4)

    e1 = nc.dram_tensor("e1", [N1, FD], f16, kind="ExternalInput")
    e2 = nc.dram_tensor("e2", [N2, FD], f16, kind="ExternalInput")
    ins = {}
    for nm_, tcn, nbt in (("g1", tc1, NBD + NBM), ("g2", tc2, NBP + NBM)):
        ins[nm_ + "_gidx"] = nc.dram_tensor(nm_ + "_gidx", [128, tcn * 8], i16,
                                            kind="ExternalInput")
        ins[nm_ + "_dest"] = nc.dram_tensor(nm_ + "_dest", [128, tcn], f16,
                                            kind="ExternalInput")
        ins[nm_ + "_vals"] = nc.dram_tensor(nm_ + "_vals", [128, tcn], f16,
                                            kind="ExternalInput")
        ins[nm_ + "_sidx"] = nc.dram_tensor(nm_ + "_sidx", [128, nbt * 8], i16,
                                            kind="ExternalInput")
    inter_d = nc.dram_tensor("inter_t", [128, 1], f32, kind="ExternalInput")
    ointer_d = nc.dram_tensor("ointer_t", [128, 1], f32, kind="ExternalInput")

    # +1 dummy row absorbs empty-slot scatter tokens (stripped on host)
    out_d = nc.dram_tensor("out_d", [ND + 1, FD], f32, kind="ExternalOutput")
    out_m = nc.dram_tensor("out_m", [NM + 1, FD], f32, kind="ExternalOutput")
    out_p = nc.dram_tensor("out_p", [NP + 1, FD], f32, kind="ExternalOutput")

    wins1, nch1 = geo1
    wins2, nch2 = geo2

    with tile.TileContext(nc) as tc:
        with (
            tc.tile_pool(name="const", bufs=1) as constp,
            tc.tile_pool(name="meta", bufs=1) as metap,
            tc.tile_pool(name="gidx", bufs=3) as gidxp,
            tc.tile_pool(name="msgs", bufs=2) as msgsp,
            tc.tile_pool(name="s", bufs=4) as sp,
            tc.tile_pool(name="res", bufs=4) as resp,
            tc.tile_pool(name="stage", bufs=2) as stagep,
            tc.tile_pool(name="psum", bufs=4, space="PSUM") as psump,
        ):
            iota_t = constp.tile([128, 128], f16)
            nc.gpsimd.iota(iota_t[:], pattern=[[1, 128]], base=0,
                           channel_multiplier=0,
                           allow_small_or_imprecise_dtypes=True)
            inter_t = constp.tile([128, 1], f32)
            nc.sync.dma_start(inter_t[:], inter_d[:])
            ointer_t = constp.tile([128, 1], f32)
            nc.sync.dma_start(ointer_t[:], ointer_d[:])

            for pair, (wins, nch, tpc, table, chb, tcn, pout, mscale) in enumerate((
                ("g1", nch1, T1, e1, CHB1, tc1, out_d, inter_t),
                ("g2", nch2, T2, e2, CHB2, tc2, out_p, ointer_t),
            )):
                nm_ = "g1" if pair == 0 else "g2"
                wins = wins1 if pair == 0 else wins2
                nbt = (NBD + NBM) if pair == 0 else (NBP + NBM)
                dest_t = metap.tile([128, tcn], f16, tag="dest")
                nc.sync.dma_start(dest_t[:], ins[nm_ + "_dest"][:])
                vals_t = metap.tile([128, tcn], f16, tag="vals")
                nc.sync.dma_start(vals_t[:], ins[nm_ + "_vals"][:])
                sidx_t = metap.tile([128, nbt * 8], i16, tag="sidx")
                nc.sync.dma_start(sidx_t[:], ins[nm_ + "_sidx"][:])

                tile_col = 0  # global tile counter (matches host stream order)
                qn = 0        # SWDGE queue round-robin
                for ph, blks in wins:
                    wb = len(blks)
                    ntw = 2 * nch * tpc * wb           # tiles in this window
                    gslab = gidxp.tile([128, ntw * 8], i16, tag="gidx")
                    nc.sync.dma_start(
                        gslab[:], ins[nm_ + "_gidx"][:, tile_col * 8:
                                                     (tile_col + ntw) * 8])
                    msgs = msgsp.tile([128, ntw, FD], f16, tag="msgs")
                    npc = 2 * tpc * wb                 # tiles per chunk gather
                    for ci in range(nch):
                        nidx = npc * 128
                        nc.gpsimd.dma_gather(
                            msgs[:, ci * npc:(ci + 1) * npc, :],
                            table[chb[ci]:chb[ci + 1], :],
                            gslab[:, ci * npc * 8:(ci + 1) * npc * 8],
                            nidx, nidx, FD, single_packet=False,
                            queue_num=qn)
                        qn = (qn + 1) % 4
                    stage = stagep.tile([128, wb, FD], f32, tag="stage")
                    for bi in range(wb):
                        res = [None, None]
                        for s2 in range(2):
                            acc = psump.tile([128, FD], f32, space="PSUM",
                                             tag="acc")
                            k = 0
                            for ci in range(nch):
                                for t in range(tpc):
                                    g = (ci * npc + s2 * tpc * wb
                                         + bi * tpc + t)
                                    col = tile_col + g
                                    s_t = sp.tile([128, 128], f16, tag="s_t")
                                    nc.vector.scalar_tensor_tensor(
                                        out=s_t[:],
                                        in0=iota_t[:],
                                        scalar=dest_t[:, col:col + 1],
                                        in1=vals_t[:, col:col + 1]
                                            .to_broadcast([128, 128]),
                                        op0=mybir.AluOpType.is_equal,
                                        op1=mybir.AluOpType.mult,
                                    )
                                    nc.tensor.matmul(
                                        out=acc[:], lhsT=s_t[:],
                                        rhs=msgs[:, g, :],
                                        start=(k == 0),
                                        stop=(k == nch * tpc - 1))
                                    k += 1
                            r = resp.tile([128, FD], f32, tag=f"res{s2}")
                            if ph == 0:
                                nc.scalar.activation(
                                    out=r[:], in_=acc[:],
                                    func=mybir.ActivationFunctionType.Relu)
                            else:
                                nc.scalar.activation(
                                    out=r[:], in_=acc[:],
                                    func=mybir.ActivationFunctionType.Relu,
                                    scale=mscale[:, :1])
                            res[s2] = r
                        nc.vector.tensor_tensor(
                            out=stage[:, bi, :], in0=res[0][:], in1=res[1][:],
                            op=mybir.AluOpType.add)
                    tgt = pout if ph == 0 else out_m
                    b0 = blks[0]
                    nc.gpsimd.dma_scatter_add(
                        tgt[:], stage[:, 0:wb, :],
                        sidx_t[:, b0 * 8:(b0 + wb) * 8],
                        wb * 128, wb * 128, FD, queue_num=qn)
                    qn = (qn + 1) % 4
                    tile_col += ntw
    nc.compile()
    return nc


def _install_ntff_shim():
    """Provide antenv.axon_hooks (missing in this image) so trace=True works."""
    import types, ctypes, contextlib
    try:
        from antenv.axon_hooks import get_axon_ntff_profile_hook  # noqa
        return
    except ImportError:
        pass
    so_path = "/opt/axon/libaxon_pjrt.so"
    try:
        lib = ctypes.CDLL(so_path)
        assert hasattr(lib, "axon_start_nrt_profile")
    except Exception:
        lib = None
    if lib is not None:
        lib.axon_start_nrt_profile.argtypes = [ctypes.POINTER(ctypes.c_int64),
                                               ctypes.c_size_t]
        lib.axon_start_nrt_profile.restype = ctypes.c_int64
        lib.axon_stop_nrt_profile.argtypes = [ctypes.c_char_p]
        lib.axon_stop_nrt_profile.restype = ctypes.c_int64

        @contextlib.contextmanager
        def _hook(output_dir, device_ids):
            import jax
            jax.devices()
            if device_ids:
                ids = (ctypes.c_int64 * len(device_ids))(*device_ids)
                rc = lib.axon_start_nrt_profile(ids, len(device_ids))
            else:
                rc = lib.axon_start_nrt_profile(None, 0)
            if rc != 0:
                raise RuntimeError(f"axon_start_nrt_profile rc={rc}")
            try:
                yield
            finally:
                n = lib.axon_stop_nrt_profile(str(output_dir).encode())
                print(f"ntff profile: {n} file(s) -> {output_dir}")
    else:
        _hook = None
    mod = types.ModuleType("antenv.axon_hooks")
    mod.get_axon_ntff_profile_hook = lambda: _hook
    mod.set_axon_ntff_profile_hook = lambda h: None
    import antenv
    sys.modules["antenv.axon_hooks"] = mod
    antenv.axon_hooks = mod


def kernel(adj1_rows, adj1_cols, adj1_vals, adj2_rows, adj2_cols, adj2_vals,
           dEmbed, mEmbed, pEmbed, inter, _trace=False):
    from concourse.bass_utils import run_bass_kernel_spmd
    if _trace:
        _install_ntff_shim()

    adj1_rows, adj1_cols = np.asarray(adj1_rows), np.asarray(adj1_cols)
    adj1_vals = np.asarray(adj1_vals)
    adj2_rows, adj2_cols = np.asarray(adj2_rows), np.asarray(adj2_cols)
    adj2_vals = np.asarray(adj2_vals)
    dEmbed = np.asarray(dEmbed, np.float32)
    mEmbed = np.asarray(mEmbed, np.float32)
    pEmbed = np.asarray(pEmbed, np.float32)
    inter_v = float(np.asarray(inter).reshape(-1)[0])

    e1 = np.concatenate([dEmbed, mEmbed], axis=0).astype(np.float16)
    e2 = np.concatenate([pEmbed, mEmbed], axis=0).astype(np.float16)

    pc1, wins1, nch1, _ = _prep_pair(
        [adj1_rows[0], adj1_rows[1]], [adj1_cols[0], adj1_cols[1]],
        [adj1_vals[0], adj1_vals[1]], DIAG, ND, NM, CHB1, T1, NBD, NBM)
    pc2, wins2, nch2, _ = _prep_pair(
        [adj2_rows[0], adj2_rows[1]], [adj2_cols[0], adj2_cols[1]],
        [adj2_vals[0], adj2_vals[1]], PRO, NP, NM, CHB2, T2, NBP, NBM)

    tc1 = pc1[0]["dest"].shape[1]
    tc2 = pc2[0]["dest"].shape[1]
    nc = _build_program((wins1, nch1), (wins2, nch2), tc1, tc2)

    inter_t = np.full((128, 1), inter_v, np.float32)
    ointer_t = np.full((128, 1), 1.0 - inter_v, np.float32)
    in_maps = []
    for cid in range(NCORES):
        in_maps.append({
            "e1": e1, "e2": e2,
            "g1_gidx": pc1[cid]["gidx"], "g1_dest": pc1[cid]["dest"],
            "g1_vals": pc1[cid]["vals"], "g1_sidx": pc1[cid]["sidx"],
            "g2_gidx": pc2[cid]["gidx"], "g2_dest": pc2[cid]["dest"],
            "g2_vals": pc2[cid]["vals"], "g2_sidx": pc2[cid]["sidx"],
            "inter_t": inter_t, "ointer_t": ointer_t,
        })

    r = run_bass_kernel_spmd(nc, in_maps, core_ids=list(range(NCORES)),
                             trace=_trace)
    outs = r.results
    dE = np.concatenate([outs[c]["out_d"][:ND] for c in range(NCORES)], axis=0)
    mE = np.concatenate([outs[c]["out_m"][:NM] for c in range(NCORES)], axis=0)
    pE = np.concatenate([outs[c]["out_p"][:NP] for c in range(NCORES)], axis=0)
    if _trace:
        return (mE, dE, pE), r
    return (mE, dE, pE)
